# revision 5
# baseline (speedup 1.0000x reference)
"""MoE positionwise FFN (SwiGLU, 7 routed experts top-2 + 1 shared) on 8 trn2 cores.

Sharding: 16382 token-FFN jobs (8192 shared + 7*1170 routed-capacity) are split
evenly: core c<7 runs routed expert c's 1170 tokens (segment A) + 878 shared
tokens (segment B); core 7 runs 2048 shared tokens (1170 + 878, last 2 padded).
Every core runs the identical SPMD Bass program; only data differs.

Routing (gate matmul + top-k + capacity selection, ~0.1% of FLOPs) runs on host
with jax-CPU mirroring the reference ops bit-for-bit. The device does the FFN
matmuls in float32r (full PE rate) in feature-major layout.
"""

import numpy as np

# Problem constants (hardcoded per task contract).
B, S, D, F, E = 4, 2048, 2048, 1024, 7
T = B * S                    # 8192 tokens
CAP = (T // E)               # 1170 capacity per expert
TOP_K = 2
TA, TB = CAP, 878            # per-core segment sizes; TA+TB = 2048
TC = TA + TB
P = 128
DB = D // P                  # 16 d-blocks
NFB = F // P                 # 8 f-pair blocks (w1 output pairs / w2 input blocks)

_PROG = None  # cached Bass program


def _chunks(Ts):
    """Split Ts tokens into near-equal moving-dim chunks, all even (fp32r ISA
    requirement) and >=256 (fp32r full-rate threshold)."""
    n = 3 if Ts == TA else 2
    c = (Ts // n + 1) // 2 * 2  # round to even
    out = []
    o = 0
    for i in range(n):
        cn = c if i < n - 1 else Ts - c * (n - 1)
        assert cn % 2 == 0 and 256 <= cn <= 512, (Ts, cn)
        out.append((o, cn))
        o += cn
    return out


def _build_program():
    from contextlib import ExitStack

    import concourse.bacc as bacc
    import concourse.mybir as mybir
    import concourse.tile as tile

    f32 = mybir.dt.float32
    f32r = mybir.dt.float32r
    ACT = mybir.ActivationFunctionType

    nc = bacc.Bacc(None, target_bir_lowering=False)

    xt = nc.dram_tensor("xt", [D, TC], f32r, kind="ExternalInput")
    w1a = nc.dram_tensor("w1a", [D, 2 * F], f32r, kind="ExternalInput")
    w2a = nc.dram_tensor("w2a", [F, D], f32r, kind="ExternalInput")
    w1b = nc.dram_tensor("w1b", [D, 2 * F], f32r, kind="ExternalInput")
    w2b = nc.dram_tensor("w2b", [F, D], f32r, kind="ExternalInput")
    yt = nc.dram_tensor("yt", [D, TC], f32, kind="ExternalOutput")

    with tile.TileContext(nc) as tc, ExitStack() as ctx:
        xt_pool = ctx.enter_context(tc.tile_pool(name="xtp", bufs=1))
        w1_pool = ctx.enter_context(tc.tile_pool(name="w1p", bufs=2))
        w2_pool = ctx.enter_context(tc.tile_pool(name="w2p", bufs=2))
        g_pool = ctx.enter_context(tc.tile_pool(name="gp", bufs=1))
        tmp_pool = ctx.enter_context(tc.tile_pool(name="tmpp", bufs=2))
        y_pool = ctx.enter_context(tc.tile_pool(name="yp", bufs=2))
        ph = ctx.enter_context(tc.tile_pool(name="ph", bufs=5, space="PSUM"))
        py = ctx.enter_context(tc.tile_pool(name="py", bufs=3, space="PSUM"))

        for w1d, w2d, t0, Ts in ((w1a, w2a, 0, TA), (w1b, w2b, TA, TB)):
            tch = _chunks(Ts)

            def load_w1pair(i, w1d=w1d):
                w1t = w1_pool.tile([P, DB, 2, P], f32r, name="w1t", tag="w1t")
                for h, col in enumerate((i, i + NFB)):
                    nc.sync.dma_start(
                        w1t[:, :, h, :],
                        w1d[:, col * P:(col + 1) * P].rearrange(
                            "(db p) f -> p db f", p=P
                        ),
                    )
                return w1t

            w1t_next = load_w1pair(0)
            xts = []
            for db in range(DB):
                xtile = xt_pool.tile([P, Ts], f32r, name=f"xts{db}", tag=f"xts{db}")
                nc.scalar.dma_start(
                    xtile[:], xt[db * P:(db + 1) * P, t0:t0 + Ts]
                )
                xts.append(xtile)

            # ---- mm1 + SwiGLU: gT[f, t] = h1 * silu(h2), f-major ----
            gts = []
            for i in range(NFB):
                w1t = w1t_next
                if i + 1 < NFB:
                    w1t_next = load_w1pair(i + 1)
                gt = g_pool.tile([P, Ts], f32r, name=f"gt{i}", tag=f"gt{i}")
                tmp = tmp_pool.tile([P, Ts], f32, name="tmp", tag="tmp")
                # x2 half (silu input): w1 cols (i+8)*128
                ps2 = [ph.tile([P, 512], f32, name="ph2", tag="ph") for _ in tch]
                for db in range(DB):
                    lhs = w1t[:, db, 1, :]
                    for ci, (c0, cn) in enumerate(tch):
                        nc.tensor.matmul(
                            ps2[ci][:, :cn],
                            lhs,
                            xts[db][:, c0:c0 + cn],
                            start=(db == 0),
                            stop=(db == DB - 1),
                        )
                for ci, (c0, cn) in enumerate(tch):
                    nc.scalar.activation(
                        tmp[:, c0:c0 + cn], ps2[ci][:, :cn], ACT.Silu
                    )
                # x1 half: w1 cols i*128
                ps1 = [ph.tile([P, 512], f32, name="ph1", tag="ph") for _ in tch]
                for db in range(DB):
                    lhs = w1t[:, db, 0, :]
                    for ci, (c0, cn) in enumerate(tch):
                        nc.tensor.matmul(
                            ps1[ci][:, :cn],
                            lhs,
                            xts[db][:, c0:c0 + cn],
                            start=(db == 0),
                            stop=(db == DB - 1),
                        )
                for ci, (c0, cn) in enumerate(tch):
                    nc.vector.tensor_mul(
                        gt[:, c0:c0 + cn], ps1[ci][:, :cn], tmp[:, c0:c0 + cn]
                    )
                gts.append(gt)

            # ---- mm2: yT[dout, t] = sum_f w2[f, dout] * gT[f, t] ----
            for do in range(DB):
                w2t = w2_pool.tile([P, NFB, P], f32r, name="w2t", tag="w2t")
                nc.sync.dma_start(
                    w2t[:],
                    w2d[:, do * P:(do + 1) * P].rearrange("(fb p) d -> p fb d", p=P),
                )
                ytsb = y_pool.tile([P, Ts], f32, name="ytsb", tag="ytsb")
                pys = [py.tile([P, 512], f32, name="py", tag="py") for _ in tch]
                for fb in range(NFB):
                    lhs = w2t[:, fb, :]
                    for ci, (c0, cn) in enumerate(tch):
                        nc.tensor.matmul(
                            pys[ci][:, :cn],
                            lhs,
                            gts[fb][:, c0:c0 + cn],
                            start=(fb == 0),
                            stop=(fb == NFB - 1),
                        )
                for ci, (c0, cn) in enumerate(tch):
                    nc.vector.tensor_copy(ytsb[:, c0:c0 + cn], pys[ci][:, :cn])
                nc.scalar.dma_start(yt[do * P:(do + 1) * P, t0:t0 + Ts], ytsb[:])

    nc.compile()
    return nc


def _get_program():
    global _PROG
    if _PROG is None:
        _PROG = _build_program()
    return _PROG


def _routing(flat_x, gate_w, expert_bias):
    """Mirror the reference gating math on jax-CPU for bit-identical selection."""
    import jax
    import jax.numpy as jnp

    cpu = jax.devices("cpu")[0]
    with jax.default_device(cpu):
        gate_logits = jnp.asarray(flat_x) @ jnp.asarray(gate_w) + jnp.asarray(
            expert_bias
        )
        aff = jax.nn.sigmoid(gate_logits)
        _, topk_idx = jax.lax.top_k(aff, TOP_K)
        mask = (topk_idx[:, :, None] == jnp.arange(E)[None, None, :]).any(axis=1)
        score = jnp.where(mask, aff, -1.0).T
        _, sel_idx = jax.lax.top_k(score, CAP)
        kept = jnp.take_along_axis(mask.T, sel_idx, axis=1)
        w = jnp.where(kept, jnp.take_along_axis(aff.T, sel_idx, axis=1), 0.0)
        sel_idx, w = np.asarray(sel_idx), np.asarray(w)
    return sel_idx, w


def _shared_slices():
    sh = [np.arange(c * TB, (c + 1) * TB) for c in range(7)]  # cores 0-6 seg B
    sh7a = np.arange(7 * TB, 7 * TB + TA)  # core 7 seg A: 6146..7315
    n7b = T - (7 * TB + TA)  # 876 real tokens in core 7 seg B
    sh7b_real = np.arange(7 * TB + TA, T)
    sh7b = np.concatenate([sh7b_real, np.zeros(TB - n7b, dtype=np.int64)])
    return sh, sh7a, sh7b_real, sh7b


def _make_in_maps(flat_x, sel_idx, shared_w1, shared_w2, routed_w1, routed_w2):
    flatT = np.ascontiguousarray(flat_x.T)  # [D, T]
    sh, sh7a, _, sh7b = _shared_slices()
    sw1 = np.ascontiguousarray(shared_w1[0])
    sw2 = np.ascontiguousarray(shared_w2[0])
    in_maps = []
    for c in range(8):
        if c < 7:
            ida, idb = sel_idx[c], sh[c]
            w1A = np.ascontiguousarray(routed_w1[c])
            w2A = np.ascontiguousarray(routed_w2[c])
        else:
            ida, idb = sh7a, sh7b
            w1A, w2A = sw1, sw2
        ids = np.concatenate([ida, idb])
        in_maps.append(
            {
                "xt": np.ascontiguousarray(flatT[:, ids]),
                "w1a": w1A,
                "w2a": w2A,
                "w1b": sw1,
                "w2b": sw2,
            }
        )
    return in_maps


def _run_device(in_maps, trace=False):
    from concourse.bass_utils import run_bass_kernel_spmd

    nc = _get_program()
    return run_bass_kernel_spmd(
        nc, in_maps, core_ids=list(range(8)), trace=trace
    )


def _combine(results, sel_idx, wgt):
    sh, sh7a, sh7b_real, _ = _shared_slices()
    out = np.zeros((T, D), np.float32)
    yts = [np.ascontiguousarray(r["yt"].T) for r in results]  # [TC, D] each
    # shared expert contributions (each token exactly once)
    for c in range(7):
        out[sh[c]] += yts[c][TA:]
    out[sh7a] += yts[7][:TA]
    out[sh7b_real] += yts[7][TA:TA + len(sh7b_real)]
    # routed contributions (indices unique within an expert)
    for c in range(7):
        out[sel_idx[c]] += yts[c][:TA] * wgt[c][:, None]
    return out


def kernel(x, gate_w, expert_bias, shared_w1, shared_w2, routed_w1, routed_w2):
    x = np.asarray(x, dtype=np.float32)
    flat_x = np.ascontiguousarray(x.reshape(T, D))
    sel_idx, wgt = _routing(flat_x, np.asarray(gate_w), np.asarray(expert_bias))
    in_maps = _make_in_maps(
        flat_x,
        sel_idx,
        np.asarray(shared_w1, dtype=np.float32),
        np.asarray(shared_w2, dtype=np.float32),
        np.asarray(routed_w1, dtype=np.float32),
        np.asarray(routed_w2, dtype=np.float32),
    )
    res = _run_device(in_maps)
    out = _combine(res.results, sel_idx, wgt)
    return out.reshape(B, S, D)
